# revision 1
# baseline (speedup 1.0000x reference)
"""Multi-head linear cross-attention Trainium2 kernel (8-core SPMD).

Math (reference):
    q    = fm(query @ Wq.T)        fm(x) = elu(x)+1 = max(x+1, min(exp(x), 1))
    gate = sigmoid(query @ Wg.T + bg)
    k, v = split(kv @ Wkv.T); k = fm(k)
    kvs  = k^T v per (b, h)   [hd, hd];  ksum = k^T 1  [hd]
    attn = (q @ kvs) / (q . ksum + eps)
    out  = (attn * gate) @ Wo.T

Sharding: kv-side split along S (each core takes S/8 rows of every batch,
computes partial [k^T v | k^T 1] summaries, AllReduce over the 8 cores);
q-side split along L (each core takes L/8 rows of every batch).

Layout: all activations are feature-major ([d, token]) on device; the host
pre-transposes/casts inputs to bf16 and re-transposes the fp32 output.
"""

import numpy as np
import ml_dtypes

import concourse.bacc as bacc
import concourse.mybir as mybir
import concourse.tile as tile
from concourse.bass_utils import run_bass_kernel_spmd

BF16 = mybir.dt.bfloat16
F32 = mybir.dt.float32
AF = mybir.ActivationFunctionType
ALU = mybir.AluOpType

EPS = 1e-6
N_CORES = 8


def build_module(B=4, L=4096, S=4096, D=2048, H=16, n_cores=N_CORES,
                 use_collective=True):
    """Emit the SPMD bass module. Requires D // H == 128."""
    HD = D // H
    assert HD == 128
    KB = D // 128            # d_in 128-blocks
    WN = 512                 # wkv streaming width
    NB = (2 * D) // WN       # wkv d_out blocks
    NBK = D // WN            # of which the first NBK are the k-projection
    S_LOC = S // n_cores     # s rows per batch per core
    MB = S_LOC // 128        # s-tiles per batch
    LT = L // n_cores        # l cols per batch per core (free dim of q-side matmuls)
    assert S_LOC % 128 == 0 and (2 * D) % WN == 0 and D % 128 == 0
    assert LT <= 512

    nc = bacc.Bacc("TRN2", target_bir_lowering=False, debug=False,
                   num_devices=n_cores)

    qT = nc.dram_tensor("qT", [D, B * LT], BF16, kind="ExternalInput")
    kvT = nc.dram_tensor("kvT", [D, B * S_LOC], BF16, kind="ExternalInput")
    wq_t = nc.dram_tensor("wq_t", [D, D], BF16, kind="ExternalInput")
    wg_t = nc.dram_tensor("wg_t", [D, D], BF16, kind="ExternalInput")
    wkv_t = nc.dram_tensor("wkv_t", [D, 2 * D], BF16, kind="ExternalInput")
    wo_t = nc.dram_tensor("wo_t", [D, D], BF16, kind="ExternalInput")
    bg_d = nc.dram_tensor("bg_d", [D, 1], F32, kind="ExternalInput")
    outT = nc.dram_tensor("outT", [D, B * LT], F32, kind="ExternalOutput")

    qT_r = qT.ap().rearrange("(k p) l -> p k l", p=128)
    kvT_r = kvT.ap().rearrange("(k p) s -> p k s", p=128)
    wq_r = wq_t.ap().rearrange("(k p) f -> p k f", p=128)
    wg_r = wg_t.ap().rearrange("(k p) f -> p k f", p=128)
    wkv_r = wkv_t.ap().rearrange("(k p) f -> p k f", p=128)
    wo_r = wo_t.ap().rearrange("(k p) f -> p k f", p=128)

    with nc.allow_low_precision(reason="bf16 matmul kernel"), \
         tile.TileContext(nc) as tc:
        with tc.tile_pool(name="const", bufs=1) as constp, \
             tc.tile_pool(name="big", bufs=1) as bigp, \
             tc.tile_pool(name="dram", bufs=1, space="DRAM") as dr:

            ones_col = constp.tile([128, 1], BF16)
            nc.vector.memset(ones_col[:], 1.0)
            ones_row = constp.tile([1, 128], BF16)
            nc.vector.memset(ones_row[:], 1.0)

            # resident activations (agf_all is allocated after phase A's
            # pool closes so phase A can use its SBUF space)
            qT_all = bigp.tile([128, KB, B * LT], BF16)
            nc.sync.dma_start(qT_all[:], qT_r)

            ar_in = [dr.tile([H, 128, 129], F32, name=f"ar_in{b}")
                     for b in range(B)]
            ar_out = [dr.tile([H, 128, 129], F32, addr_space="Shared",
                              name=f"ar_out{b}") for b in range(B)]

            # ---------------- Phase A: kv projection + summaries ----------
            with tc.tile_pool(name="sbA", bufs=2) as sba, \
                 tc.tile_pool(name="psA", bufs=2, space="PSUM") as psa:
                for b in range(B):
                    kvT_b = sba.tile([128, KB, S_LOC], BF16, tag="kvT_b",
                                     bufs=1, name=f"kvT_b{b}")
                    nc.sync.dma_start(kvT_b[:],
                                      kvT_r[:, :, b * S_LOC:(b + 1) * S_LOC])
                    k_sb = [sba.tile([128, D], BF16, tag=f"k_sb{m}", bufs=1,
                                     name=f"k_sb_{b}_{m}") for m in range(MB)]
                    v_sb = [sba.tile([128, D], BF16, tag=f"v_sb{m}", bufs=1,
                                     name=f"v_sb_{b}_{m}") for m in range(MB)]
                    for n in range(NB):
                        wkv_n = sba.tile([128, KB, WN], BF16, tag="wkv_n",
                                         bufs=2, name=f"wkv_n_{b}_{n}")
                        nc.sync.dma_start(wkv_n[:],
                                          wkv_r[:, :, n * WN:(n + 1) * WN])
                        for m in range(MB):
                            kvp_ps = psa.tile([128, WN], F32, tag="kvp_ps",
                                              bufs=3, name=f"kvp_ps_{b}_{n}_{m}")
                            for k in range(KB):
                                nc.tensor.matmul(
                                    kvp_ps[:],
                                    kvT_b[:, k, m * 128:(m + 1) * 128],
                                    wkv_n[:, k, :],
                                    start=(k == 0), stop=(k == KB - 1))
                            if n < NBK:
                                # fm(x) = max(x + 1, min(exp(x), 1))
                                e_sb = sba.tile([128, WN], F32, tag="e_sb",
                                                bufs=2, name=f"e_sb_{b}_{n}_{m}")
                                nc.scalar.activation(e_sb[:], kvp_ps[:], AF.Exp)
                                nc.vector.tensor_scalar_min(e_sb[:], e_sb[:], 1.0)
                                nc.vector.scalar_tensor_tensor(
                                    k_sb[m][:, n * WN:(n + 1) * WN],
                                    kvp_ps[:], 1.0, e_sb[:], ALU.add, ALU.max)
                            else:
                                nc.scalar.copy(
                                    v_sb[m][:, (n - NBK) * WN:(n - NBK + 1) * WN],
                                    kvp_ps[:])
                    for h in range(H):
                        kvs_ps = psa.tile([128, 128], F32, tag="kvs_ps",
                                          bufs=2, name=f"kvs_ps_{b}_{h}")
                        ks_ps = psa.tile([128, 1], F32, tag="ks_ps",
                                         bufs=2, name=f"ks_ps_{b}_{h}")
                        for m in range(MB):
                            nc.tensor.matmul(
                                kvs_ps[:],
                                k_sb[m][:, h * 128:(h + 1) * 128],
                                v_sb[m][:, h * 128:(h + 1) * 128],
                                start=(m == 0), stop=(m == MB - 1))
                        for m in range(MB):
                            nc.tensor.matmul(
                                ks_ps[:],
                                k_sb[m][:, h * 128:(h + 1) * 128],
                                ones_col[:],
                                start=(m == 0), stop=(m == MB - 1))
                        kvs_cp = sba.tile([128, 129], F32, tag="kvs_cp",
                                          bufs=2, name=f"kvs_cp_{b}_{h}")
                        nc.scalar.copy(kvs_cp[:, 0:128], kvs_ps[:])
                        nc.scalar.copy(kvs_cp[:, 128:129], ks_ps[:])
                        nc.sync.dma_start(ar_in[b][h], kvs_cp[:])
                    # reduce this batch's summaries while later batches compute
                    if use_collective:
                        nc.gpsimd.collective_compute(
                            "AllReduce", ALU.add,
                            replica_groups=[list(range(n_cores))],
                            ins=[ar_in[b].opt()], outs=[ar_out[b].opt()])
                    else:
                        nc.sync.dma_start(ar_out[b][:], ar_in[b][:])

            agf_all, _agf_free = tc.tile([128, H, B * LT], BF16,
                                         name="agf_all")

            # ---------------- Phase B: q/gate proj + attention -------------
            with tc.tile_pool(name="sbB", bufs=2) as sbb, \
                 tc.tile_pool(name="psB", bufs=2, space="PSUM") as psb:
                for h in range(H):
                    wq_h = sbb.tile([128, KB, 128], BF16, tag="wq_h", bufs=2,
                                    name=f"wq_h_{h}")
                    nc.sync.dma_start(wq_h[:], wq_r[:, :, h * 128:(h + 1) * 128])
                    wg_h = sbb.tile([128, KB, 128], BF16, tag="wg_h", bufs=2,
                                    name=f"wg_h_{h}")
                    nc.sync.dma_start(wg_h[:], wg_r[:, :, h * 128:(h + 1) * 128])
                    bg_h = sbb.tile([128, 1], F32, tag="bg_h", bufs=2,
                                    name=f"bg_h_{h}")
                    nc.sync.dma_start(bg_h[:], bg_d.ap()[h * 128:(h + 1) * 128, :])
                    for b in range(B):
                        kvs_f = sbb.tile([128, 129], F32, tag="kvs_f", bufs=2,
                                         name=f"kvs_f_{h}_{b}")
                        nc.sync.dma_start(kvs_f[:], ar_out[b][h])
                        kvs_bf = sbb.tile([128, 129], BF16, tag="kvs_bf", bufs=2,
                                          name=f"kvs_bf_{h}_{b}")
                        nc.vector.tensor_copy(kvs_bf[:], kvs_f[:])

                        lsl = slice(b * LT, (b + 1) * LT)
                        q_ps = psb.tile([128, LT], F32, tag="q_ps", bufs=2,
                                        name=f"q_ps_{h}_{b}")
                        for k in range(KB):
                            nc.tensor.matmul(q_ps[:], wq_h[:, k, :],
                                             qT_all[:, k, lsl],
                                             start=(k == 0), stop=(k == KB - 1))
                        e2_sb = sbb.tile([128, LT], F32, tag="e2_sb", bufs=2,
                                         name=f"e2_sb_{h}_{b}")
                        nc.scalar.activation(e2_sb[:], q_ps[:], AF.Exp)
                        nc.vector.tensor_scalar_min(e2_sb[:], e2_sb[:], 1.0)
                        qfm = sbb.tile([128, LT], BF16, tag="qfm", bufs=2,
                                       name=f"qfm_{h}_{b}")
                        nc.vector.scalar_tensor_tensor(
                            qfm[:], q_ps[:], 1.0, e2_sb[:], ALU.add, ALU.max)

                        g_ps = psb.tile([128, LT], F32, tag="g_ps", bufs=2,
                                        name=f"g_ps_{h}_{b}")
                        for k in range(KB):
                            nc.tensor.matmul(g_ps[:], wg_h[:, k, :],
                                             qT_all[:, k, lsl],
                                             start=(k == 0), stop=(k == KB - 1))
                        gate_sb = sbb.tile([128, LT], BF16, tag="gate_sb",
                                           bufs=2, name=f"gate_sb_{h}_{b}")
                        nc.scalar.activation(gate_sb[:], g_ps[:], AF.Sigmoid,
                                             bias=bg_h[:])

                        att_ps = psb.tile([128, LT], F32, tag="att_ps", bufs=2,
                                          name=f"att_ps_{h}_{b}")
                        nc.tensor.matmul(att_ps[:], kvs_bf[:, 0:128], qfm[:],
                                         start=True, stop=True)
                        den_ps = psb.tile([1, LT], F32, tag="den_ps", bufs=1,
                                          name=f"den_ps_{h}_{b}")
                        nc.tensor.matmul(den_ps[:], kvs_bf[:, 128:129], qfm[:],
                                         start=True, stop=True)
                        den_sb = sbb.tile([1, LT], F32, tag="den_sb", bufs=2,
                                          name=f"den_sb_{h}_{b}")
                        nc.scalar.copy(den_sb[:], den_ps[:])
                        nc.vector.tensor_scalar_add(den_sb[:], den_sb[:], EPS)
                        rden_sb = sbb.tile([1, LT], BF16, tag="rden_sb", bufs=2,
                                           name=f"rden_sb_{h}_{b}")
                        nc.vector.reciprocal(rden_sb[:], den_sb[:])
                        bc_ps = psb.tile([128, LT], F32, tag="bc_ps", bufs=1,
                                         name=f"bc_ps_{h}_{b}")
                        nc.tensor.matmul(bc_ps[:], ones_row[:], rden_sb[:],
                                         start=True, stop=True)
                        g2_sb = sbb.tile([128, LT], BF16, tag="g2_sb", bufs=2,
                                         name=f"g2_sb_{h}_{b}")
                        nc.vector.tensor_tensor(g2_sb[:], gate_sb[:], bc_ps[:],
                                                ALU.mult)
                        nc.vector.tensor_tensor(agf_all[:, h, lsl], att_ps[:],
                                                g2_sb[:], ALU.mult)

            # ---------------- Phase C: output projection -------------------
            with tc.tile_pool(name="sbC", bufs=2) as sbc, \
                 tc.tile_pool(name="psC", bufs=2, space="PSUM") as psc:
                for do in range(KB):
                    wo_do = sbc.tile([128, KB, 128], BF16, tag="wo_do", bufs=2,
                                     name=f"wo_do_{do}")
                    nc.sync.dma_start(wo_do[:],
                                      wo_r[:, :, do * 128:(do + 1) * 128])
                    for b in range(B):
                        lsl = slice(b * LT, (b + 1) * LT)
                        o_ps = psc.tile([128, LT], F32, tag="o_ps", bufs=2,
                                        name=f"o_ps_{do}_{b}")
                        for hh in range(H):
                            nc.tensor.matmul(o_ps[:], wo_do[:, hh, :],
                                             agf_all[:, hh, lsl],
                                             start=(hh == 0),
                                             stop=(hh == H - 1))
                        ot_sb = sbc.tile([128, LT], F32, tag="ot_sb", bufs=2,
                                         name=f"ot_sb_{do}_{b}")
                        nc.scalar.copy(ot_sb[:], o_ps[:])
                        nc.sync.dma_start(
                            outT.ap()[do * 128:(do + 1) * 128, lsl], ot_sb[:])

            _agf_free()

    nc.compile()
    return nc


def prep_in_maps(query, kv, Wq, Wg, bg, Wkv, Wo, n_cores=N_CORES):
    B, L, D = query.shape
    S = kv.shape[1]
    LT = L // n_cores
    S_LOC = S // n_cores
    bf = ml_dtypes.bfloat16

    wq_t = np.ascontiguousarray(np.asarray(Wq).T).astype(bf)
    wg_t = np.ascontiguousarray(np.asarray(Wg).T).astype(bf)
    wkv_t = np.ascontiguousarray(np.asarray(Wkv).T).astype(bf)
    wo_t = np.ascontiguousarray(np.asarray(Wo).T).astype(bf)
    bg_d = np.ascontiguousarray(np.asarray(bg, dtype=np.float32).reshape(D, 1))
    query = np.asarray(query)
    kv = np.asarray(kv)

    in_maps = []
    for c in range(n_cores):
        qs = query[:, c * LT:(c + 1) * LT, :]          # [B, LT, D]
        qT_c = np.ascontiguousarray(
            qs.transpose(2, 0, 1).reshape(D, B * LT)).astype(bf)
        ks = kv[:, c * S_LOC:(c + 1) * S_LOC, :]       # [B, S_LOC, D]
        kvT_c = np.ascontiguousarray(
            ks.transpose(2, 0, 1).reshape(D, B * S_LOC)).astype(bf)
        in_maps.append({
            "qT": qT_c, "kvT": kvT_c,
            "wq_t": wq_t, "wg_t": wg_t, "wkv_t": wkv_t, "wo_t": wo_t,
            "bg_d": bg_d,
        })
    return in_maps


def assemble_output(results, B, L, D, n_cores=N_CORES):
    LT = L // n_cores
    out = np.empty((B, L, D), np.float32)
    for c in range(n_cores):
        outT = results[c]["outT"]                      # [D, B*LT]
        per = outT.reshape(D, B, LT)
        out[:, c * LT:(c + 1) * LT, :] = per.transpose(1, 2, 0)
    return out


_NC_CACHE = {}


def _get_module(key):
    if key not in _NC_CACHE:
        B, L, S, D, H = key
        _NC_CACHE[key] = build_module(B=B, L=L, S=S, D=D, H=H)
    return _NC_CACHE[key]


def kernel(query, kv, Wq, Wg, bg, Wkv, Wo):
    query = np.asarray(query)
    kv = np.asarray(kv)
    B, L, D = query.shape
    S = kv.shape[1]
    H = 16
    nc = _get_module((B, L, S, D, H))
    in_maps = prep_in_maps(query, kv, Wq, Wg, bg, Wkv, Wo)
    res = run_bass_kernel_spmd(nc, in_maps, core_ids=list(range(N_CORES)))
    return assemble_output(res.results, B, L, D)

